# revision 48
# baseline (speedup 1.0000x reference)
"""Trainium2 Bass kernel for nn_MultiHeadAttention_80418967650946.

Reference computation (per batch b):
  qp/kp/vp = 1x1-conv projections of q/k/v   [64, N]
  funky head view: qh[h,n,d] = qp.reshape(4, 16*N)[d, 16n+h]  (same for kh, vh)
  scores = qh @ kh * 0.25^0.5 + bias ; attn = softmax(scores)
  x[4h+d, n] = (attn @ vh)[h, n, d] ; y = LeakyReLU(BN(Wo @ x + bo), 0.2)

Sharding: 8 cores = 4 batches x 2 query-halves (n in [0,512) or [512,1024)).
Each core computes its query-half for ALL 16 heads fully locally (no
collectives): the output conv is column-wise independent, so y[:, n-half]
only needs x[:, n-half].

Per-core device algorithm (all matmul accum fp32).  The elementwise
softmax work (exp + bias apply) is the throughput limiter, so it is
split per 256-key chunk u of each head so ScalarE and DVE carry
balanced shares:
  - u in {0,1}: ScalarE exp(s)->bf16, then one DVE bf16 2x-rate
    multiply by host-precomputed exp(b)/32 for both chunks at once.
  - u = 3 (plus u = 2 on even heads): one DVE scalar_tensor_tensor
    computes i16 = round(alpha*s + eb16) where eb16 = round(alpha*b + C)
    is a host-precomputed int16; the i16 bit pattern IS bf16(exp(s+b)/32)
    (Schraudolph bit-trick, +-3% relerr).  No ScalarE work.
  - u = 2 on odd heads takes the ScalarE path instead, balancing the
    two engines at ~3.3/~3.7 us per head.
  The /32 scaling cancels in the softmax normalization.
attn@V lags the scores pipeline by three heads so short producer
hiccups never stall the PE queue head.

attn@V packs FOUR heads into one [128,512] psum bank: per head an M=8
matmul (4 v-cols + 4 ones-cols at PE column-position 32*(h%4)) so rows
32g+0..3 hold x and rows 32g+4..7 hold the softmax denominator; these
matmuls run concurrently with the next head's qk matmuls (disjoint PE
column groups).  Per 4-head group, one psum->sbuf bulk copy (ScalarE)
plus strided DMAs gather x / denominator rows into packed [32,512]
accumulators; one reciprocal and one multiply per half normalize 8
heads at a time (reciprocal_approx_fast only works at partition base 0).

The BN scale is folded into Wo host-side and the BN bias rides the
output conv as a 65th ones-row of x, so the conv psum holds the final
pre-activation and the tail is just LeakyReLU (one DVE op per half).
"""
import sys

if "/opt/trn_rl_repo" not in sys.path:
    sys.path.insert(0, "/opt/trn_rl_repo")

import numpy as np
import ml_dtypes

import concourse.bass as bass
import concourse.tile as tile
from concourse import bacc, mybir
from concourse.bass_utils import run_bass_kernel_spmd

F32 = mybir.dt.float32
I16 = mybir.dt.int16
AF = mybir.ActivationFunctionType
ALU = mybir.AluOpType
PSUM = bass.MemorySpace.PSUM
F32R = mybir.dt.float32r
BF16 = mybir.dt.bfloat16


H = 16
D = 4
HID = 256
B = 4
N = 1024
NH = 512          # per-core query positions
NCORES = 8
SCALE = float(D) ** -0.5
BN_EPS = 1e-5
NEG_SLOPE = 0.2
ALPHA = 128.0 * float(np.log2(np.e))   # 184.6627...
C_SCH = 128.0 * 122.0 - 5.5            # 15610.5 (Schraudolph offset incl. /32)
NEG_LN32 = -float(np.log(32.0))


def _emit(nc, tc, io):
    qkvb, wqkv = io["qkvb"], io["wqkv"]
    eball, woT = io["eball"], io["woT"]
    y = io["y"]

    with (
        tc.tile_pool(name="persist", bufs=1) as persist,
        tc.tile_pool(name="bias", bufs=4) as bp,
        tc.tile_pool(name="emul", bufs=10) as em,
        tc.tile_pool(name="exp", bufs=3) as ep,
        tc.tile_pool(name="sml", bufs=4) as sp,
        tc.tile_pool(name="p1", bufs=1) as p1,
        tc.tile_pool(name="ps_s", bufs=3, space=PSUM) as pss,
        tc.tile_pool(name="ps_x", bufs=2, space=PSUM) as psx,
    ):
        # Ks[u][32g + d, 64h + i] = kp-channel (16d + 4u + g) at position
        # 16i + h  ==  kh[h, d, m] for m = 64*(4u+g) + i  (head-contiguous
        # so the scores LDWEIGHTS reads are contiguous)
        Ks = [persist.tile([128, N], BF16, tag=f"Ks{u}", name=f"Ks{u}")
              for u in range(4)]
        Qp2 = persist.tile([100, H * NH], BF16, tag="Qp2")
        # Vdr[p, (h, u, v2, c8)]: c8 in 0..3 = vh[m = 256u+128v2+p, d]; 4..7 = 1.0
        Vdr = persist.tile([128, H * 64], BF16, tag="Vdr")
        # packed attn@V results: per half (A = heads 0..7, B = heads 8..15)
        xuA = persist.tile([32, NH], F32, tag="xuA")
        xuB = persist.tile([32, NH], F32, tag="xuB")
        dnA = persist.tile([32, NH], F32, tag="dnA")
        dnB = persist.tile([32, NH], F32, tag="dnB")
        rcpA = persist.tile([32, NH], F32, tag="rcpA")
        rcpB = persist.tile([32, NH], F32, tag="rcpB")
        x_sb = persist.tile([65, NH], BF16, tag="x_sb")
        woT_sb = persist.tile([65, HID], BF16, tag="woT_sb")

        # ---- PE warm-up + ACT table preload: no input deps, issue at t=0.
        wu_w = p1.tile([128, 128], BF16, tag="wu_w")
        wu_r = p1.tile([128, 512], BF16, tag="wu_r")
        zt = p1.tile([128, 128], BF16, tag="zt")
        nc.vector.memset(wu_w[:], 0.03125)
        nc.vector.memset(wu_r[:], 0.03125)
        nc.vector.memset(zt[:], 0.0)
        scr = p1.tile([128, 8], F32, tag="scr")
        nc.scalar.activation(scr[:], wu_w[:, 0:8], AF.Exp)
        nc.vector.memset(x_sb[:], 1.0)      # row 64 stays 1.0 (BN-bias row)
        ps_w = pss.tile([128, 512], F32, tag="ps")
        for i in range(10):
            nc.tensor.matmul(ps_w[:], wu_w[:], wu_r[:],
                             start=(i == 0), stop=(i == 9))

        # ---------------- phase 1: input DMAs ----------------
        # weights on gpsimd (tiny, gate the projections); bulk q/k/v on the
        # sync HWDGE queue; the first bias prefetches go BEHIND qkv on sync
        # so they don't steal HBM bandwidth from the critical-path inputs.
        w_sb = p1.tile([128, 1216], BF16, tag="w_sb")
        nc.gpsimd.dma_start(w_sb[:].rearrange("p (c o) -> p c o", c=2),
                            wqkv.rearrange("(c p) o -> p c o", p=128))
        qkv_sb = p1.tile([128, 6144], BF16, tag="qkv_sb")
        nc.sync.dma_start(qkv_sb[:, 2048:4096].rearrange("p (c n) -> p c n", c=2),
                          qkvb[256:512].rearrange("(c p) n -> p c n", p=128))
        nc.sync.dma_start(qkv_sb[:, 0:2048].rearrange("p (c n) -> p c n", c=2),
                          qkvb[0:256].rearrange("(c p) n -> p c n", p=128))
        nc.sync.dma_start(qkv_sb[:, 4096:6144].rearrange("p (c n) -> p c n", c=2),
                          qkvb[512:768].rearrange("(c p) n -> p c n", p=128))
        nc.gpsimd.dma_start(woT_sb[:], woT)
        q_sb = qkv_sb[:, 0:2048]
        k_sb = qkv_sb[:, 2048:4096]
        v_sb = qkv_sb[:, 4096:6144]

        # bias prefetch: one [128, 4096] (8 KiB/partition contiguous) per head
        bias_tiles = {}

        def fetch_bias(h):
            bt = bp.tile([128, 4096], BF16, tag="bh")
            if h < 3:
                nc.sync.dma_start(bt[:], eball[h])
            else:
                nc.gpsimd.dma_start(bt[:], eball[h])
            bias_tiles[h] = bt

        for h in range(3):
            fetch_bias(h)

        # ---------------- K projection (first: scores need Ks earliest) ----
        # M=32 matmuls with host-zero-padded lhsT -> fully-initialized psum,
        # one bulk [100,1024] strided copy stages tile b4 into Ks[b4].
        def kproj(b4):
            psk = pss.tile([128, 1024], F32, tag="ps")
            for g in range(4):
                j = 4 * b4 + g
                for nn2 in range(2):
                    for c in range(2):
                        nc.tensor.matmul(
                            psk[32 * g:32 * g + 32, 512 * nn2:512 * nn2 + 512],
                            w_sb[:, 608 * c + 32 + 32 * j:608 * c + 64 + 32 * j],
                            k_sb[:, 1024 * c + 512 * nn2:1024 * c + 512 * nn2 + 512],
                            start=(c == 0), stop=(c == 1), tile_position=(0, 32 * g))
            dstv = Ks[b4][0:100, :].rearrange("p (h i) -> p i h", h=16)
            srcv = psk[0:100, :].rearrange("p (i h) -> p i h", i=64)
            if b4 % 2 == 1:
                nc.scalar.copy(dstv, srcv)
            else:
                nc.vector.tensor_copy(dstv, srcv)

        kproj(0)
        kproj(1)

        # ---------------- Q projection ----------------
        # 4 j-values col-tiled per [128,1024] psum tile (rows 32g+d hold
        # j = 4*b4+g); SCALE is folded into Wq host-side so the head-major
        # gather into Qp2 is a plain strided copy, split DVE/ScalarE.
        for b4 in range(2):
            psq = pss.tile([128, 1024], F32, tag="ps")
            for g in range(4):
                j = 4 * b4 + g
                for nn2 in range(2):
                    for c in range(2):
                        nc.tensor.matmul(
                            psq[32 * g:32 * g + 4, 512 * nn2:512 * nn2 + 512],
                            w_sb[:, 608 * c + 4 * j:608 * c + 4 * j + 4],
                            q_sb[:, 1024 * c + 512 * nn2:1024 * c + 512 * nn2 + 512],
                            start=(c == 0), stop=(c == 1), tile_position=(0, 32 * g))
            for g in range(4):
                j = 4 * b4 + g
                srcv = psq[32 * g:32 * g + 4, :].rearrange("d (a b) -> d b a", b=16)
                dstv = Qp2[0:4, :].rearrange("d (b q) -> d b q", b=16)[:, :, 64 * j:64 * j + 64]
                if g % 2 == 0:
                    nc.vector.tensor_copy(dstv, srcv)
                else:
                    nc.scalar.copy(dstv, srcv)
        for rep in range(1, 4):
            nc.sync.dma_start(Qp2[32 * rep:32 * rep + 4, :], Qp2[0:4, :])

        kproj(2)
        kproj(3)

        # ---------------- phase 2 stage functions ----------------
        def scores_tile(h, u, ps=None, start=True):
            """qk matmuls for key chunk u of head h -> psum tile.
            psum[64*mh + i, 512*v2 + n] = s(m = 256u + 128v2 + 64mh + i, n)."""
            if ps is None:
                ps = pss.tile([128, 1024], F32, tag="ps")
            for v2 in range(2):
                for mh in range(2):
                    rg = 2 * v2 + mh
                    nc.tensor.matmul(
                        ps[64 * mh:64 * mh + 64, 512 * v2:512 * v2 + 512],
                        Ks[u][32 * rg:32 * rg + 4, 64 * h:64 * h + 64],
                        Qp2[32 * rg:32 * rg + 4, 512 * h:512 * h + 512],
                        start=start, stop=True,
                        tile_position=(32 * rg, 64 * mh))
            return ps

        def head_scores_P0(h, bt):
            """Chunks u=0,1: ScalarE exp(s) then one DVE 2x-rate bf16
            multiply by host-precomputed exp(b)/32."""
            ex0 = ep.tile([128, 2048], BF16, tag="ex")
            ems0 = em.tile([128, 2048], BF16, tag="ems")
            for uu in range(2):
                ps = scores_tile(h, uu)
                nc.scalar.activation(ex0[:, 1024 * uu:1024 * uu + 1024], ps[:], AF.Exp)
            nc.vector.tensor_mul(ems0[:], ex0[:], bt[:, 0:2048])
            return ems0

        def head_scores_P1(h, bt):
            """Chunk u=3 (and u=2 on even heads) -> DVE Schraudolph
            bit-trick exp; u=2 on odd heads AND the last four heads ->
            ScalarE exp + DVE mul, so ScalarE and DVE carry balanced
            steady-state shares and the DVE queue drains before the
            final attn@V instead of stalling it."""
            ems1 = em.tile([128, 2048], BF16, tag="ems")
            if h % 2 == 1 or h >= 8:
                ps = scores_tile(h, 2)
                ex1 = ep.tile([128, 1024], BF16, tag="ex1")
                nc.scalar.activation(ex1[:], ps[:], AF.Exp)
                nc.vector.tensor_mul(ems1[:, 0:1024], ex1[:], bt[:, 2048:3072])
            else:
                ps = scores_tile(h, 2)
                nc.vector.scalar_tensor_tensor(
                    ems1[:, 0:1024].bitcast(I16), ps[:], ALPHA,
                    bt[:, 2048:3072].bitcast(I16), ALU.mult, ALU.add)
            ps = scores_tile(h, 3)
            nc.vector.scalar_tensor_tensor(
                ems1[:, 1024:2048].bitcast(I16), ps[:], ALPHA,
                bt[:, 3072:4096].bitcast(I16), ALU.mult, ALU.add)
            return ems1

        def head_scores(h):
            if h + 2 not in bias_tiles and h + 2 < H:
                fetch_bias(h + 2)
            bt = bias_tiles.pop(h)
            return [head_scores_P0(h, bt), head_scores_P1(h, bt)]

        Vw = Vdr[:].rearrange("p (h u v c) -> p h u v c", u=4, v=2, c=8)

        def attnv(st, part):
            """part 0: chunks u=0,1; part 1: chunks u=2,3 (psum group tile pt).
            Two zero-weight ballast matmuls per part keep the PE issue rate
            just below the exp-producer rate so it never stalls (a stalled
            PE re-throttles the HAM clock gate to 1.2 GHz)."""
            h, ems, pt = st
            g = h % 4
            for uu in range(2):
                u = 2 * part + uu
                for v2 in range(2):
                    nc.tensor.matmul(
                        pt[32 * g:32 * g + 8, :],
                        Vw[:, h, u, v2, :],
                        ems[part][:, 1024 * uu + 512 * v2:1024 * uu + 512 * v2 + 512],
                        start=(u == 0 and v2 == 0), stop=(u == 3 and v2 == 1),
                        tile_position=(0, 32 * g))

        def group_gather(t, pt, gs=(0, 1, 2, 3)):
            """After heads 4t+gs accumulated into pt, stage psum to sbuf
            (ScalarE) and DMA-gather x rows (32g+0..3) / denom rows
            (32g+4..7) into the packed accumulators."""
            g0, g1 = gs[0], gs[-1]
            xg = sp.tile([104, NH], F32, tag="xg", name=f"xg{t}_{g0}")
            nc.scalar.copy(xg[32 * g0:32 * g1 + 8, :], pt[32 * g0:32 * g1 + 8, :])
            xuT = xuA if t < 2 else xuB
            dnT = dnA if t < 2 else dnB
            for g in gs:
                rd = (16 * t + 4 * g) % 32
                if t == 3 and g % 2 == 1:
                    nc.gpsimd.dma_start(xuT[rd:rd + 4, :], xg[32 * g:32 * g + 4, :])
                    nc.gpsimd.dma_start(dnT[rd:rd + 4, :], xg[32 * g + 4:32 * g + 8, :])
                else:
                    nc.sync.dma_start(xuT[rd:rd + 4, :], xg[32 * g:32 * g + 4, :])
                    nc.sync.dma_start(dnT[rd:rd + 4, :], xg[32 * g + 4:32 * g + 8, :])

        # heads 0 and 1's scores are emitted BEFORE the V projection: their
        # exps keep ScalarE/DVE busy while the V projection runs on PE.
        ems_h0 = head_scores(0)
        ems_h1 = head_scores(1)

        # ---------------- V projection ----------------
        # One [128, 512] psum tile: psV[64*half + i, 64*s2 + 4*c2 + d]
        #   = vp-channel (16d + c2) at position (16i + 2*s2 + half)
        #   = vh[head 2*s2+half, m = 64*c2 + i, d]
        psV = psx.tile([128, 512], F32, tag="psx")
        for s2 in range(8):
            for half in range(2):
                for c in range(2):
                    nc.tensor.matmul(
                        psV[64 * half:64 * half + 64, 64 * s2:64 * s2 + 64],
                        v_sb[:, 1024 * c + 2 * s2 + half:1024 * c + 2 * s2 + half + 1009:16],
                        w_sb[:, 608 * c + 544:608 * c + 608],
                        start=(c == 0), stop=(c == 1),
                        tile_position=(0, 64 * half))
        nc.vector.memset(Vdr[:], 1.0)
        # stage into Vdr: head h = 2*s2 + Hh, m = 64*c2 + i, p = 64*(c2%2) + i,
        # and the (u, v2) pair index w = c2 // 2 directly.
        for Hh in range(2):
            for par in range(2):
                srcv = psV[64 * Hh:64 * Hh + 64, :].rearrange(
                    "i (s c d) -> i s c d", s=8, c=16)[:, :, par:16:2, :]
                dstv = Vdr[64 * par:64 * par + 64, :].rearrange(
                    "p (h w c) -> p h w c", w=8, c=8)[:, Hh:H:2, :, 0:4]
                if par == 0:
                    nc.vector.tensor_copy(dstv, srcv)
                else:
                    nc.scalar.copy(dstv, srcv)

        # ---------------- phase 2: attention ----------------
        # attn@V lags the scores pipeline by TWO heads so the PE never
        # waits on the exp production (ScalarE/DVE) at a head boundary.
        heads = {0: (0, ems_h0), 1: (1, ems_h1)}
        heads[2] = (2, head_scores(2))
        pts = {}
        for h in range(3, H + 3):
            hv = h - 3             # head whose attn@V we emit this iteration
            t, g = divmod(hv, 4)
            if g == 0:
                pts[t] = psx.tile([128, NH], F32, tag="psx", name=f"pt{t}")
            if h < H:
                heads[h] = (h, head_scores(h))
            st = (hv, heads.pop(hv)[1], pts[t])
            attnv(st, 0)
            attnv(st, 1)
            if g == 3 and t < 3:
                group_gather(t, pts.pop(t))
                if t == 1:
                    # heads 0..7 normalized off the critical tail
                    nc.vector.reciprocal_approx_fast(rcpA[:], dnA[:])
                    nc.vector.tensor_mul(x_sb[0:32, :], xuA[:], rcpA[:])
            if hv == 13:
                group_gather(3, pts[3], gs=(0, 1))
        group_gather(3, pts.pop(3), gs=(2, 3))

        # ---------------- normalize + output conv + LeakyReLU --------------
        # BN scale is folded into woT host-side; BN bias rides as x-row 64
        # (ones): the conv psum holds the final pre-activation.  Zero-weight
        # ballast matmuls keep the PE array busy (HAM warm) across the
        # gather/normalize latency before the conv.
        psys = []
        for u in range(2):
            psy = pss.tile([128, NH], F32, tag="ps")
            for i in range(5):
                nc.tensor.matmul(psy[:], zt[:], wu_r[:],
                                 start=(i == 0), stop=False, tile_position=(0, 0))
            psys.append(psy)
        nc.vector.reciprocal_approx_fast(rcpB[:], dnB[:])
        nc.vector.tensor_mul(x_sb[32:64, :], xuB[:], rcpB[:])
        for u in range(2):
            psy = psys[u]
            nc.tensor.matmul(psy[:], woT_sb[0:65, 128 * u:128 * u + 128], x_sb[:],
                             start=False, stop=True)
            z = sp.tile([128, NH], F32, tag="z")
            nc.scalar.copy(z[:], psy[:])
            yt = sp.tile([128, NH], F32, tag="yt")
            nc.vector.scalar_tensor_tensor(yt[:], psy[:], NEG_SLOPE, z[:],
                                           ALU.mult, ALU.max)
            nc.sync.dma_start(y[128 * u:128 * u + 128, :], yt[:])

        if "dbg_x" in io:
            nc.sync.dma_start(io["dbg_x"], x_sb[:])
            nc.sync.dma_start(io["dbg_vdr"], Vdr[:])
            nc.sync.dma_start(io["dbg_dn"][0:32], dnA[:])
            nc.sync.dma_start(io["dbg_dn"][32:64], dnB[:])
            nc.sync.dma_start(io["dbg_xu"][0:32], xuA[:])
            nc.sync.dma_start(io["dbg_xu"][32:64], xuB[:])


def build_program(debug_outputs=False):
    nc = bacc.Bacc("TRN2", target_bir_lowering=False, debug=False)
    io = {
        "qkvb": nc.dram_tensor("qkvb", [3 * HID, N], BF16, kind="ExternalInput").ap(),
        "eball": nc.dram_tensor("eball", [H, 128, 4096], BF16, kind="ExternalInput").ap(),
        "wqkv": nc.dram_tensor("wqkv", [HID, 608], BF16, kind="ExternalInput").ap(),
        "woT": nc.dram_tensor("woT", [65, HID], BF16, kind="ExternalInput").ap(),
        "y": nc.dram_tensor("y", [HID, NH], F32, kind="ExternalOutput").ap(),
    }
    if debug_outputs:
        io["dbg_x"] = nc.dram_tensor("dbg_x", [65, NH], BF16, kind="ExternalOutput").ap()
        io["dbg_vdr"] = nc.dram_tensor("dbg_vdr", [128, H * 64], BF16, kind="ExternalOutput").ap()
        io["dbg_dn"] = nc.dram_tensor("dbg_dn", [64, NH], F32, kind="ExternalOutput").ap()
        io["dbg_xu"] = nc.dram_tensor("dbg_xu", [64, NH], F32, kind="ExternalOutput").ap()
    with tile.TileContext(nc) as tc:
        _emit(nc, tc, io)
    nc.compile()
    return nc


def make_in_maps(q, k, v, attn_bias, Wq, Wk, Wv, Wo, bo, gamma, beta, run_mean, run_var):
    def f32(x):
        return np.ascontiguousarray(np.asarray(x, dtype=np.float32))

    def b16(x):
        return np.ascontiguousarray(np.asarray(x, dtype=np.float32).astype(ml_dtypes.bfloat16))

    q, k, v, attn_bias = f32(q), f32(k), f32(v), f32(attn_bias)
    Wq, Wk, Wv, Wo, bo = f32(Wq), f32(Wk), f32(Wv), f32(Wo), f32(bo)
    gamma, beta, run_mean, run_var = f32(gamma), f32(beta), f32(run_mean), f32(run_var)

    # zero-padded K weight layout: col 32*j + r holds Wk row (j + 16*r)
    # for r < 4, zeros elsewhere -> the M=32 projection matmuls fully
    # initialize their psum row-groups.
    wk3 = np.zeros((HID, 512), dtype=np.float32)
    for j in range(16):
        for r in range(4):
            wk3[:, 32 * j + r] = Wk[j + 16 * r, :]
    # V weights with cols (c2, d): col 4*c2 + d = Wv row (16*d + c2)
    wv2 = np.empty((HID, 64), dtype=np.float32)
    for c2 in range(16):
        for d in range(4):
            wv2[:, 4 * c2 + d] = Wv[16 * d + c2, :]
    # output conv with BN scale folded in and BN bias as row 64
    s = (gamma / np.sqrt(run_var + BN_EPS))
    t = (bo - run_mean) * s + beta
    woT = np.empty((65, HID), dtype=np.float32)
    woT[0:64] = (Wo * s[:, None]).T
    woT[64] = t

    in_maps = []
    for core in range(NCORES):
        b, half = divmod(core, 2)
        n0 = half * NH
        rows = np.array([16 * d + 8 * half + jl for jl in range(8) for d in range(4)])
        wqT = Wq[rows, :].T * SCALE                               # [256, 32], col = 4*jl+d
        wqkv = b16(np.concatenate([wqT, wk3, wv2], axis=1))       # [256, 608]
        qkvb = b16(np.concatenate([q[b], k[b], v[b]], axis=0))    # [768, 1024]
        # bias tensor: T[h, u, p, 512*v2 + n] = b[b, h, n0+n, 256u+128v2+p]
        bt = attn_bias[b, :, n0:n0 + NH, :]                       # [16, 512n, 1024m]
        T = bt.reshape(H, NH, 4, 2, 128).transpose(0, 2, 4, 3, 1).reshape(H, 4, 128, N)
        bits = np.empty((H, 4, 128, N), np.uint16)
        bits[:, :2] = np.asarray(np.exp(T[:, :2]) / 32.0,
                                 dtype=ml_dtypes.bfloat16).view(np.uint16)
        bits[:, 2:] = np.round(T[:, 2:] * ALPHA + C_SCH).astype(np.int16).view(np.uint16)
        act_u2 = [h for h in range(H) if h % 2 == 1 or h >= 8]
        bits[act_u2, 2] = np.asarray(np.exp(T[act_u2, 2]) / 32.0,
                                     dtype=ml_dtypes.bfloat16).view(np.uint16)
        eball = np.ascontiguousarray(
            bits.transpose(0, 2, 1, 3).reshape(H, 128, 4096)).view(ml_dtypes.bfloat16)
        in_maps.append({
            "qkvb": qkvb, "eball": eball, "wqkv": wqkv, "woT": b16(woT),
        })
    return in_maps


_NC_CACHE = None


def get_nc():
    global _NC_CACHE
    if _NC_CACHE is None:
        _NC_CACHE = build_program()
    return _NC_CACHE


def kernel(**inputs):
    nc = get_nc()
    in_maps = make_in_maps(**inputs)
    res = run_bass_kernel_spmd(nc, in_maps, list(range(NCORES)))
    out = np.empty((B, HID, N), dtype=np.float32)
    for core in range(NCORES):
        b, half = divmod(core, 2)
        out[b, :, half * NH:(half + 1) * NH] = res.results[core]["y"]
    return out


# revision 49
# speedup vs baseline: 1.0040x; 1.0040x over previous
"""Trainium2 Bass kernel for nn_MultiHeadAttention_80418967650946.

Reference computation (per batch b):
  qp/kp/vp = 1x1-conv projections of q/k/v   [64, N]
  funky head view: qh[h,n,d] = qp.reshape(4, 16*N)[d, 16n+h]  (same for kh, vh)
  scores = qh @ kh * 0.25^0.5 + bias ; attn = softmax(scores)
  x[4h+d, n] = (attn @ vh)[h, n, d] ; y = LeakyReLU(BN(Wo @ x + bo), 0.2)

Sharding: 8 cores = 4 batches x 2 query-halves (n in [0,512) or [512,1024)).
Each core computes its query-half for ALL 16 heads fully locally (no
collectives): the output conv is column-wise independent, so y[:, n-half]
only needs x[:, n-half].

Per-core device algorithm (all matmul accum fp32).  The elementwise
softmax work (exp + bias apply) is the throughput limiter, so it is
split per 256-key chunk u of each head so ScalarE and DVE carry
balanced shares:
  - u in {0,1}: ScalarE exp(s)->bf16, then one DVE bf16 2x-rate
    multiply by host-precomputed exp(b)/32 for both chunks at once.
  - u = 3 (plus u = 2 on even heads): one DVE scalar_tensor_tensor
    computes i16 = round(alpha*s + eb16) where eb16 = round(alpha*b + C)
    is a host-precomputed int16; the i16 bit pattern IS bf16(exp(s+b)/32)
    (Schraudolph bit-trick, +-3% relerr).  No ScalarE work.
  - u = 2 on odd heads takes the ScalarE path instead, balancing the
    two engines at ~3.3/~3.7 us per head.
  The /32 scaling cancels in the softmax normalization.
attn@V lags the scores pipeline by three heads so short producer
hiccups never stall the PE queue head.

attn@V packs FOUR heads into one [128,512] psum bank: per head an M=8
matmul (4 v-cols + 4 ones-cols at PE column-position 32*(h%4)) so rows
32g+0..3 hold x and rows 32g+4..7 hold the softmax denominator; these
matmuls run concurrently with the next head's qk matmuls (disjoint PE
column groups).  Per 4-head group, one psum->sbuf bulk copy (ScalarE)
plus strided DMAs gather x / denominator rows into packed [32,512]
accumulators; one reciprocal and one multiply per half normalize 8
heads at a time (reciprocal_approx_fast only works at partition base 0).

The BN scale is folded into Wo host-side and the BN bias rides the
output conv as a 65th ones-row of x, so the conv psum holds the final
pre-activation and the tail is just LeakyReLU (one DVE op per half).
"""
import sys

if "/opt/trn_rl_repo" not in sys.path:
    sys.path.insert(0, "/opt/trn_rl_repo")

import numpy as np
import ml_dtypes

import concourse.bass as bass
import concourse.tile as tile
from concourse import bacc, mybir
from concourse.bass_utils import run_bass_kernel_spmd

F32 = mybir.dt.float32
I16 = mybir.dt.int16
AF = mybir.ActivationFunctionType
ALU = mybir.AluOpType
PSUM = bass.MemorySpace.PSUM
F32R = mybir.dt.float32r
BF16 = mybir.dt.bfloat16


H = 16
D = 4
HID = 256
B = 4
N = 1024
NH = 512          # per-core query positions
NCORES = 8
SCALE = float(D) ** -0.5
BN_EPS = 1e-5
NEG_SLOPE = 0.2
ALPHA = 128.0 * float(np.log2(np.e))   # 184.6627...
C_SCH = 128.0 * 122.0 - 5.5            # 15610.5 (Schraudolph offset incl. /32)
NEG_LN32 = -float(np.log(32.0))


def _emit(nc, tc, io):
    qkvb, wqkv = io["qkvb"], io["wqkv"]
    eball, woT = io["eball"], io["woT"]
    y = io["y"]

    with (
        tc.tile_pool(name="persist", bufs=1) as persist,
        tc.tile_pool(name="bias", bufs=4) as bp,
        tc.tile_pool(name="emul", bufs=10) as em,
        tc.tile_pool(name="exp", bufs=3) as ep,
        tc.tile_pool(name="sml", bufs=6) as sp,
        tc.tile_pool(name="p1", bufs=1) as p1,
        tc.tile_pool(name="ps_s", bufs=3, space=PSUM) as pss,
        tc.tile_pool(name="ps_x", bufs=2, space=PSUM) as psx,
    ):
        # Ks[u][32g + d, 64h + i] = kp-channel (16d + 4u + g) at position
        # 16i + h  ==  kh[h, d, m] for m = 64*(4u+g) + i  (head-contiguous
        # so the scores LDWEIGHTS reads are contiguous)
        Ks = [persist.tile([128, N], BF16, tag=f"Ks{u}", name=f"Ks{u}")
              for u in range(4)]
        Qp2 = persist.tile([100, H * NH], BF16, tag="Qp2")
        # Vdr[p, (h, u, v2, c8)]: c8 in 0..3 = vh[m = 256u+128v2+p, d]; 4..7 = 1.0
        Vdr = persist.tile([128, H * 64], BF16, tag="Vdr")
        # packed attn@V results: per half (A = heads 0..7, B = heads 8..15)
        xuA = persist.tile([32, NH], F32, tag="xuA")
        xuB = persist.tile([32, NH], F32, tag="xuB")
        dnA = persist.tile([32, NH], F32, tag="dnA")
        dnB = persist.tile([32, NH], F32, tag="dnB")
        rcpA = persist.tile([32, NH], F32, tag="rcpA")
        rcpB = persist.tile([32, NH], F32, tag="rcpB")
        x_sb = persist.tile([65, NH], BF16, tag="x_sb")
        woT_sb = persist.tile([65, HID], BF16, tag="woT_sb")

        # ---- PE warm-up + ACT table preload: no input deps, issue at t=0.
        wu_w = p1.tile([128, 128], BF16, tag="wu_w")
        wu_r = p1.tile([128, 512], BF16, tag="wu_r")
        zt = p1.tile([128, 128], BF16, tag="zt")
        nc.vector.memset(wu_w[:], 0.03125)
        nc.vector.memset(wu_r[:], 0.03125)
        nc.vector.memset(zt[:], 0.0)
        scr = p1.tile([128, 8], F32, tag="scr")
        nc.scalar.activation(scr[:], wu_w[:, 0:8], AF.Exp)
        nc.vector.memset(x_sb[:], 1.0)      # row 64 stays 1.0 (BN-bias row)
        ps_w = pss.tile([128, 512], F32, tag="ps")
        for i in range(5):
            nc.tensor.matmul(ps_w[:], wu_w[:], wu_r[:],
                             start=(i == 0), stop=(i == 4))

        # ---------------- phase 1: input DMAs ----------------
        # weights on gpsimd (tiny, gate the projections); bulk q/k/v on the
        # sync HWDGE queue; the first bias prefetches go BEHIND qkv on sync
        # so they don't steal HBM bandwidth from the critical-path inputs.
        w_sb = p1.tile([128, 1216], BF16, tag="w_sb")
        nc.gpsimd.dma_start(w_sb[:].rearrange("p (c o) -> p c o", c=2),
                            wqkv.rearrange("(c p) o -> p c o", p=128))
        qkv_sb = p1.tile([128, 6144], BF16, tag="qkv_sb")
        nc.sync.dma_start(qkv_sb[:, 2048:4096].rearrange("p (c n) -> p c n", c=2),
                          qkvb[256:512].rearrange("(c p) n -> p c n", p=128))
        nc.sync.dma_start(qkv_sb[:, 0:2048].rearrange("p (c n) -> p c n", c=2),
                          qkvb[0:256].rearrange("(c p) n -> p c n", p=128))
        nc.sync.dma_start(qkv_sb[:, 4096:6144].rearrange("p (c n) -> p c n", c=2),
                          qkvb[512:768].rearrange("(c p) n -> p c n", p=128))
        nc.gpsimd.dma_start(woT_sb[:], woT)
        q_sb = qkv_sb[:, 0:2048]
        k_sb = qkv_sb[:, 2048:4096]
        v_sb = qkv_sb[:, 4096:6144]

        # bias prefetch: one [128, 4096] (8 KiB/partition contiguous) per head
        bias_tiles = {}

        def fetch_bias(h):
            bt = bp.tile([128, 4096], BF16, tag="bh")
            if h < 3:
                nc.sync.dma_start(bt[:], eball[h])
            else:
                nc.gpsimd.dma_start(bt[:], eball[h])
            bias_tiles[h] = bt

        for h in range(3):
            fetch_bias(h)

        # ---------------- K projection (first: scores need Ks earliest) ----
        # M=32 matmuls with host-zero-padded lhsT -> fully-initialized psum,
        # one bulk [100,1024] strided copy stages tile b4 into Ks[b4].
        def kproj(b4):
            psk = pss.tile([128, 1024], F32, tag="ps")
            for g in range(4):
                j = 4 * b4 + g
                for nn2 in range(2):
                    for c in range(2):
                        nc.tensor.matmul(
                            psk[32 * g:32 * g + 32, 512 * nn2:512 * nn2 + 512],
                            w_sb[:, 608 * c + 32 + 32 * j:608 * c + 64 + 32 * j],
                            k_sb[:, 1024 * c + 512 * nn2:1024 * c + 512 * nn2 + 512],
                            start=(c == 0), stop=(c == 1), tile_position=(0, 32 * g))
            dstv = Ks[b4][0:100, :].rearrange("p (h i) -> p i h", h=16)
            srcv = psk[0:100, :].rearrange("p (i h) -> p i h", i=64)
            if b4 % 2 == 1:
                nc.scalar.copy(dstv, srcv)
            else:
                nc.vector.tensor_copy(dstv, srcv)

        kproj(0)
        kproj(1)

        # ---------------- Q projection ----------------
        # 4 j-values col-tiled per [128,1024] psum tile (rows 32g+d hold
        # j = 4*b4+g); SCALE is folded into Wq host-side so the head-major
        # gather into Qp2 is a plain strided copy, split DVE/ScalarE.
        for b4 in range(2):
            psq = pss.tile([128, 1024], F32, tag="ps")
            for g in range(4):
                j = 4 * b4 + g
                for nn2 in range(2):
                    for c in range(2):
                        nc.tensor.matmul(
                            psq[32 * g:32 * g + 4, 512 * nn2:512 * nn2 + 512],
                            w_sb[:, 608 * c + 4 * j:608 * c + 4 * j + 4],
                            q_sb[:, 1024 * c + 512 * nn2:1024 * c + 512 * nn2 + 512],
                            start=(c == 0), stop=(c == 1), tile_position=(0, 32 * g))
            for g in range(4):
                j = 4 * b4 + g
                srcv = psq[32 * g:32 * g + 4, :].rearrange("d (a b) -> d b a", b=16)
                dstv = Qp2[0:4, :].rearrange("d (b q) -> d b q", b=16)[:, :, 64 * j:64 * j + 64]
                if g % 2 == 0:
                    nc.vector.tensor_copy(dstv, srcv)
                else:
                    nc.scalar.copy(dstv, srcv)
        for rep in range(1, 4):
            nc.sync.dma_start(Qp2[32 * rep:32 * rep + 4, :], Qp2[0:4, :])

        kproj(2)
        kproj(3)

        # ---------------- phase 2 stage functions ----------------
        def scores_tile(h, u, ps=None, start=True):
            """qk matmuls for key chunk u of head h -> psum tile.
            psum[64*mh + i, 512*v2 + n] = s(m = 256u + 128v2 + 64mh + i, n)."""
            if ps is None:
                ps = pss.tile([128, 1024], F32, tag="ps")
            for v2 in range(2):
                for mh in range(2):
                    rg = 2 * v2 + mh
                    nc.tensor.matmul(
                        ps[64 * mh:64 * mh + 64, 512 * v2:512 * v2 + 512],
                        Ks[u][32 * rg:32 * rg + 4, 64 * h:64 * h + 64],
                        Qp2[32 * rg:32 * rg + 4, 512 * h:512 * h + 512],
                        start=start, stop=True,
                        tile_position=(32 * rg, 64 * mh))
            return ps

        def head_scores_P0(h, bt):
            """Chunks u=0,1: ScalarE exp(s) then one DVE 2x-rate bf16
            multiply by host-precomputed exp(b)/32."""
            ex0 = ep.tile([128, 2048], BF16, tag="ex")
            ems0 = em.tile([128, 2048], BF16, tag="ems")
            for uu in range(2):
                ps = scores_tile(h, uu)
                nc.scalar.activation(ex0[:, 1024 * uu:1024 * uu + 1024], ps[:], AF.Exp)
            nc.vector.tensor_mul(ems0[:], ex0[:], bt[:, 0:2048])
            return ems0

        def head_scores_P1(h, bt):
            """Chunk u=3 (and u=2 on even heads) -> DVE Schraudolph
            bit-trick exp; u=2 on odd heads AND the last four heads ->
            ScalarE exp + DVE mul, so ScalarE and DVE carry balanced
            steady-state shares and the DVE queue drains before the
            final attn@V instead of stalling it."""
            ems1 = em.tile([128, 2048], BF16, tag="ems")
            if h % 2 == 1 or h >= 8:
                ps = scores_tile(h, 2)
                ex1 = ep.tile([128, 1024], BF16, tag="ex1")
                nc.scalar.activation(ex1[:], ps[:], AF.Exp)
                nc.vector.tensor_mul(ems1[:, 0:1024], ex1[:], bt[:, 2048:3072])
            else:
                ps = scores_tile(h, 2)
                nc.vector.scalar_tensor_tensor(
                    ems1[:, 0:1024].bitcast(I16), ps[:], ALPHA,
                    bt[:, 2048:3072].bitcast(I16), ALU.mult, ALU.add)
            ps = scores_tile(h, 3)
            nc.vector.scalar_tensor_tensor(
                ems1[:, 1024:2048].bitcast(I16), ps[:], ALPHA,
                bt[:, 3072:4096].bitcast(I16), ALU.mult, ALU.add)
            return ems1

        def head_scores(h):
            if h + 2 not in bias_tiles and h + 2 < H:
                fetch_bias(h + 2)
            bt = bias_tiles.pop(h)
            return [head_scores_P0(h, bt), head_scores_P1(h, bt)]

        Vw = Vdr[:].rearrange("p (h u v c) -> p h u v c", u=4, v=2, c=8)

        def attnv(st, part):
            """part 0: chunks u=0,1; part 1: chunks u=2,3 (psum group tile pt).
            Two zero-weight ballast matmuls per part keep the PE issue rate
            just below the exp-producer rate so it never stalls (a stalled
            PE re-throttles the HAM clock gate to 1.2 GHz)."""
            h, ems, pt = st
            g = h % 4
            for uu in range(2):
                u = 2 * part + uu
                for v2 in range(2):
                    nc.tensor.matmul(
                        pt[32 * g:32 * g + 8, :],
                        Vw[:, h, u, v2, :],
                        ems[part][:, 1024 * uu + 512 * v2:1024 * uu + 512 * v2 + 512],
                        start=(u == 0 and v2 == 0), stop=(u == 3 and v2 == 1),
                        tile_position=(0, 32 * g))

        def group_gather(t, pt, gs=(0, 1, 2, 3)):
            """After heads 4t+gs accumulated into pt, stage psum to sbuf
            (ScalarE) and DMA-gather x rows (32g+0..3) / denom rows
            (32g+4..7) into the packed accumulators."""
            g0, g1 = gs[0], gs[-1]
            xg = sp.tile([104, NH], F32, tag="xg", name=f"xg{t}_{g0}")
            nc.scalar.copy(xg[32 * g0:32 * g1 + 8, :], pt[32 * g0:32 * g1 + 8, :])
            xuT = xuA if t < 2 else xuB
            dnT = dnA if t < 2 else dnB
            for g in gs:
                rd = (16 * t + 4 * g) % 32
                if t == 3 and g % 2 == 1:
                    nc.gpsimd.dma_start(xuT[rd:rd + 4, :], xg[32 * g:32 * g + 4, :])
                    nc.gpsimd.dma_start(dnT[rd:rd + 4, :], xg[32 * g + 4:32 * g + 8, :])
                else:
                    nc.sync.dma_start(xuT[rd:rd + 4, :], xg[32 * g:32 * g + 4, :])
                    nc.sync.dma_start(dnT[rd:rd + 4, :], xg[32 * g + 4:32 * g + 8, :])

        # heads 0 and 1's scores are emitted BEFORE the V projection: their
        # exps keep ScalarE/DVE busy while the V projection runs on PE.
        ems_h0 = head_scores(0)
        ems_h1 = head_scores(1)

        # ---------------- V projection ----------------
        # One [128, 512] psum tile: psV[64*half + i, 64*s2 + 4*c2 + d]
        #   = vp-channel (16d + c2) at position (16i + 2*s2 + half)
        #   = vh[head 2*s2+half, m = 64*c2 + i, d]
        psV = psx.tile([128, 512], F32, tag="psx")
        for s2 in range(8):
            for half in range(2):
                for c in range(2):
                    nc.tensor.matmul(
                        psV[64 * half:64 * half + 64, 64 * s2:64 * s2 + 64],
                        v_sb[:, 1024 * c + 2 * s2 + half:1024 * c + 2 * s2 + half + 1009:16],
                        w_sb[:, 608 * c + 544:608 * c + 608],
                        start=(c == 0), stop=(c == 1),
                        tile_position=(0, 64 * half))
        nc.vector.memset(Vdr[:], 1.0)
        # stage into Vdr: head h = 2*s2 + Hh, m = 64*c2 + i, p = 64*(c2%2) + i,
        # and the (u, v2) pair index w = c2 // 2 directly.
        for Hh in range(2):
            for par in range(2):
                srcv = psV[64 * Hh:64 * Hh + 64, :].rearrange(
                    "i (s c d) -> i s c d", s=8, c=16)[:, :, par:16:2, :]
                dstv = Vdr[64 * par:64 * par + 64, :].rearrange(
                    "p (h w c) -> p h w c", w=8, c=8)[:, Hh:H:2, :, 0:4]
                if par == 0:
                    nc.vector.tensor_copy(dstv, srcv)
                else:
                    nc.scalar.copy(dstv, srcv)

        # ---------------- phase 2: attention ----------------
        # attn@V lags the scores pipeline by TWO heads so the PE never
        # waits on the exp production (ScalarE/DVE) at a head boundary.
        heads = {0: (0, ems_h0), 1: (1, ems_h1)}
        heads[2] = (2, head_scores(2))
        pts = {}
        for h in range(3, H + 3):
            hv = h - 3             # head whose attn@V we emit this iteration
            t, g = divmod(hv, 4)
            if g == 0:
                pts[t] = psx.tile([128, NH], F32, tag="psx", name=f"pt{t}")
            if h < H:
                heads[h] = (h, head_scores(h))
            st = (hv, heads.pop(hv)[1], pts[t])
            attnv(st, 0)
            attnv(st, 1)
            if g == 3 and t < 3:
                group_gather(t, pts.pop(t))
                if t == 1:
                    # heads 0..7 normalized off the critical tail
                    nc.vector.reciprocal_approx_fast(rcpA[:], dnA[:])
                    nc.vector.tensor_mul(x_sb[0:32, :], xuA[:], rcpA[:])
            if hv == 13:
                group_gather(3, pts[3], gs=(0, 1))
        group_gather(3, pts.pop(3), gs=(2, 3))

        # ---------------- normalize + output conv + LeakyReLU --------------
        # BN scale is folded into woT host-side; BN bias rides as x-row 64
        # (ones): the conv psum holds the final pre-activation.  Zero-weight
        # ballast matmuls keep the PE array busy (HAM warm) across the
        # gather/normalize latency before the conv.
        psys = []
        for u in range(2):
            psy = pss.tile([128, NH], F32, tag="ps")
            for i in range(5):
                nc.tensor.matmul(psy[:], zt[:], wu_r[:],
                                 start=(i == 0), stop=False, tile_position=(0, 0))
            psys.append(psy)
        nc.vector.reciprocal_approx_fast(rcpB[:], dnB[:])
        nc.vector.tensor_mul(x_sb[32:64, :], xuB[:], rcpB[:])
        for u in range(2):
            psy = psys[u]
            nc.tensor.matmul(psy[:], woT_sb[0:65, 128 * u:128 * u + 128], x_sb[:],
                             start=False, stop=True)
            z = sp.tile([128, NH], F32, tag="z")
            nc.scalar.copy(z[:], psy[:])
            yt = sp.tile([128, NH], F32, tag="yt")
            nc.vector.scalar_tensor_tensor(yt[:], psy[:], NEG_SLOPE, z[:],
                                           ALU.mult, ALU.max)
            nc.sync.dma_start(y[128 * u:128 * u + 128, :], yt[:])

        if "dbg_x" in io:
            nc.sync.dma_start(io["dbg_x"], x_sb[:])
            nc.sync.dma_start(io["dbg_vdr"], Vdr[:])
            nc.sync.dma_start(io["dbg_dn"][0:32], dnA[:])
            nc.sync.dma_start(io["dbg_dn"][32:64], dnB[:])
            nc.sync.dma_start(io["dbg_xu"][0:32], xuA[:])
            nc.sync.dma_start(io["dbg_xu"][32:64], xuB[:])


def build_program(debug_outputs=False):
    nc = bacc.Bacc("TRN2", target_bir_lowering=False, debug=False)
    io = {
        "qkvb": nc.dram_tensor("qkvb", [3 * HID, N], BF16, kind="ExternalInput").ap(),
        "eball": nc.dram_tensor("eball", [H, 128, 4096], BF16, kind="ExternalInput").ap(),
        "wqkv": nc.dram_tensor("wqkv", [HID, 608], BF16, kind="ExternalInput").ap(),
        "woT": nc.dram_tensor("woT", [65, HID], BF16, kind="ExternalInput").ap(),
        "y": nc.dram_tensor("y", [HID, NH], F32, kind="ExternalOutput").ap(),
    }
    if debug_outputs:
        io["dbg_x"] = nc.dram_tensor("dbg_x", [65, NH], BF16, kind="ExternalOutput").ap()
        io["dbg_vdr"] = nc.dram_tensor("dbg_vdr", [128, H * 64], BF16, kind="ExternalOutput").ap()
        io["dbg_dn"] = nc.dram_tensor("dbg_dn", [64, NH], F32, kind="ExternalOutput").ap()
        io["dbg_xu"] = nc.dram_tensor("dbg_xu", [64, NH], F32, kind="ExternalOutput").ap()
    with tile.TileContext(nc) as tc:
        _emit(nc, tc, io)
    nc.compile()
    return nc


def make_in_maps(q, k, v, attn_bias, Wq, Wk, Wv, Wo, bo, gamma, beta, run_mean, run_var):
    def f32(x):
        return np.ascontiguousarray(np.asarray(x, dtype=np.float32))

    def b16(x):
        return np.ascontiguousarray(np.asarray(x, dtype=np.float32).astype(ml_dtypes.bfloat16))

    q, k, v, attn_bias = f32(q), f32(k), f32(v), f32(attn_bias)
    Wq, Wk, Wv, Wo, bo = f32(Wq), f32(Wk), f32(Wv), f32(Wo), f32(bo)
    gamma, beta, run_mean, run_var = f32(gamma), f32(beta), f32(run_mean), f32(run_var)

    # zero-padded K weight layout: col 32*j + r holds Wk row (j + 16*r)
    # for r < 4, zeros elsewhere -> the M=32 projection matmuls fully
    # initialize their psum row-groups.
    wk3 = np.zeros((HID, 512), dtype=np.float32)
    for j in range(16):
        for r in range(4):
            wk3[:, 32 * j + r] = Wk[j + 16 * r, :]
    # V weights with cols (c2, d): col 4*c2 + d = Wv row (16*d + c2)
    wv2 = np.empty((HID, 64), dtype=np.float32)
    for c2 in range(16):
        for d in range(4):
            wv2[:, 4 * c2 + d] = Wv[16 * d + c2, :]
    # output conv with BN scale folded in and BN bias as row 64
    s = (gamma / np.sqrt(run_var + BN_EPS))
    t = (bo - run_mean) * s + beta
    woT = np.empty((65, HID), dtype=np.float32)
    woT[0:64] = (Wo * s[:, None]).T
    woT[64] = t

    in_maps = []
    for core in range(NCORES):
        b, half = divmod(core, 2)
        n0 = half * NH
        rows = np.array([16 * d + 8 * half + jl for jl in range(8) for d in range(4)])
        wqT = Wq[rows, :].T * SCALE                               # [256, 32], col = 4*jl+d
        wqkv = b16(np.concatenate([wqT, wk3, wv2], axis=1))       # [256, 608]
        qkvb = b16(np.concatenate([q[b], k[b], v[b]], axis=0))    # [768, 1024]
        # bias tensor: T[h, u, p, 512*v2 + n] = b[b, h, n0+n, 256u+128v2+p]
        bt = attn_bias[b, :, n0:n0 + NH, :]                       # [16, 512n, 1024m]
        T = bt.reshape(H, NH, 4, 2, 128).transpose(0, 2, 4, 3, 1).reshape(H, 4, 128, N)
        bits = np.empty((H, 4, 128, N), np.uint16)
        bits[:, :2] = np.asarray(np.exp(T[:, :2]) / 32.0,
                                 dtype=ml_dtypes.bfloat16).view(np.uint16)
        bits[:, 2:] = np.round(T[:, 2:] * ALPHA + C_SCH).astype(np.int16).view(np.uint16)
        act_u2 = [h for h in range(H) if h % 2 == 1 or h >= 8]
        bits[act_u2, 2] = np.asarray(np.exp(T[act_u2, 2]) / 32.0,
                                     dtype=ml_dtypes.bfloat16).view(np.uint16)
        eball = np.ascontiguousarray(
            bits.transpose(0, 2, 1, 3).reshape(H, 128, 4096)).view(ml_dtypes.bfloat16)
        in_maps.append({
            "qkvb": qkvb, "eball": eball, "wqkv": wqkv, "woT": b16(woT),
        })
    return in_maps


_NC_CACHE = None


def get_nc():
    global _NC_CACHE
    if _NC_CACHE is None:
        _NC_CACHE = build_program()
    return _NC_CACHE


def kernel(**inputs):
    nc = get_nc()
    in_maps = make_in_maps(**inputs)
    res = run_bass_kernel_spmd(nc, in_maps, list(range(NCORES)))
    out = np.empty((B, HID, N), dtype=np.float32)
    for core in range(NCORES):
        b, half = divmod(core, 2)
        out[b, :, half * NH:(half + 1) * NH] = res.results[core]["y"]
    return out
